# revision 55
# baseline (speedup 1.0000x reference)
"""Bayesian MLP MC-sample kernel for one TRN2 chip (8 NeuronCores).

Problem: out[s, b, o] for S=32 MC samples of a 3-layer MLP
  dims 256 -> 512 -> 512 -> 64, batch B=2048,
  w_s = z_w[s] * exp(w_log_std) + w_mean   (per-sample reparameterized weights)
  h1 = tanh(x @ w0_s + b0_s); h2 = tanh(h1 @ w1_s + b1_s); out = h2 @ w2_s + b2_s

Sharding: MC-sample axis across the 8 cores (4 samples/core); x and the
mean/log_std parameters are replicated. No cross-core communication.

On-chip layout: everything stays transposed (features on partitions,
batch on the free dim) so the matmul contraction is always the partition
dim and no transposes are needed on device:
  h^T[dout, B] = W^T x^T   via  matmul(psum, lhsT=w[k_chunk, dout_chunk],
                                       rhs=hprev^T[k_chunk, batch_slice])
The host passes x^T in and transposes the (S, 64, B) result back to
(S, B, 64) when gathering -- pure layout work, no FLOPs.

Per-core engine usage:
  PE:  4 samples x 112 matmuls (N=512, f32r -> 1 cycle/row)
  ACT: exp(log_std) once; per-sample tanh(psum + bias) eviction
  DVE: per-sample w = z * sigma + mean (two passes, in place)
  DMA: z shards + replicated params + x^T in, out^T back
"""

import numpy as np

import concourse.bass as bass
import concourse.mybir as mybir
import concourse.tile as tile
from concourse import bacc
from concourse import bass_utils

F32 = mybir.dt.float32
F32R = mybir.dt.float32r
# matmul compute dtype: f32r (TF32-like, 1 cycle/row at N>=256) gives
# 2.4e-4 end-to-end rel err vs 3.9e-3 for bf16 at only ~4% more runtime
MMDT = F32R
AF = mybir.ActivationFunctionType
ts = bass.ts

S = 32
B = 2048
DIMS = [256, 512, 512, 64]
NCORES = 8
SL = S // NCORES  # samples per core
NSLICE = 512      # moving-dim slice (max for fp32 matmul, = 1 PSUM bank)
NB = B // NSLICE

# knobs test.py may override before the first kernel() call
RUN_KWARGS: dict = {}
LAST_RESULT = None

_CACHE: dict = {}


def _build_nc():
    nc = bacc.Bacc("TRN2", target_bir_lowering=False)

    xT = nc.dram_tensor("xT", [DIMS[0], B], F32, kind="ExternalInput")
    w_mean, w_ls, b_mean, b_ls, z_w, z_b = [], [], [], [], [], []
    for li in range(3):
        din, dout = DIMS[li], DIMS[li + 1]
        w_mean.append(nc.dram_tensor(f"w_mean_{li}", [din, dout], F32, kind="ExternalInput"))
        w_ls.append(nc.dram_tensor(f"w_log_std_{li}", [din, dout], F32, kind="ExternalInput"))
        b_mean.append(nc.dram_tensor(f"b_mean_{li}", [dout], F32, kind="ExternalInput"))
        b_ls.append(nc.dram_tensor(f"b_log_std_{li}", [dout], F32, kind="ExternalInput"))
        z_w.append(nc.dram_tensor(f"z_w_{li}", [SL, din, dout], F32, kind="ExternalInput"))
        z_b.append(nc.dram_tensor(f"z_b_{li}", [SL, dout], F32, kind="ExternalInput"))
    out_d = nc.dram_tensor("out", [SL, DIMS[3], B], F32, kind="ExternalOutput")

    NK = [d // 128 for d in DIMS[:3]]      # k-chunks per layer: 2, 4, 4
    MP = [min(128, d) for d in DIMS[1:]]   # psum partitions:  128, 128, 64
    NM = [d // 128 if d >= 128 else 1 for d in DIMS[1:]]  # m-chunks: 4, 4, 1
    BP = [min(128, d) for d in DIMS[1:]]   # bias partitions
    BC = [max(1, d // 128) for d in DIMS[1:]]  # bias cols

    with tile.TileContext(nc) as tc:
        with (
            tc.tile_pool(name="const", bufs=1) as cpool,
            tc.tile_pool(name="z", bufs=2) as zpool,
            tc.tile_pool(name="w0", bufs=2) as w0p,
            tc.tile_pool(name="w1", bufs=2) as w1p,
            tc.tile_pool(name="w2", bufs=2) as w2p,
            tc.tile_pool(name="h1", bufs=2) as h1p,
            tc.tile_pool(name="h2", bufs=1) as h2p,
            tc.tile_pool(name="osb", bufs=1) as opool,
            tc.tile_pool(name="ps", bufs=2, space="PSUM") as pspool,
        ):
            wpools = [w0p, w1p, w2p]
            # All transfers ride the sync-engine HWDGE ring: a large
            # dma_start spreads over all 16 SDMA engines (~350 GB/s), and
            # the ring serves transfers in issue order, which doubles as
            # the prefetch priority. (Measured: gpsimd SWDGE ~20 GB/s and
            # scalar-ring issues stall the ACT FIFO -- both hurt; small
            # transfers cost ~1.5-2 us of ring dead time each, hence the
            # per-layer batching of bias vectors.)
            hw1 = nc.sync
            sw = nc.sync

            sigma = [None] * 3
            mean = [None] * 3
            sigma_b = [None] * 3
            mean_b = [None] * 3

            ball = [None] * 3

            def emit_bias_layer(li, dma=None):
                # all SL samples' bias noise in ONE transfer -- small DMAs
                # cost ~1.5 us of ring dead time each
                dma = dma or sw
                bp, bc = BP[li], BC[li]
                bz = cpool.tile([bp, SL, bc], F32, tag=f"ball{li}")
                dma.dma_start(bz[:], z_b[li][:].rearrange("s (c p) -> p s c", p=bp))
                ball[li] = bz
                sgb = cpool.tile([bp, bc], F32, tag=f"sigma_b{li}")
                dma.dma_start(sgb[:], b_ls[li][:].rearrange("(c p) -> p c", p=bp))
                nc.scalar.activation(sgb[:], sgb[:], AF.Exp)
                sigma_b[li] = sgb
                mnb = cpool.tile([bp, bc], F32, tag=f"mean_b{li}")
                dma.dma_start(mnb[:], b_mean[li][:].rearrange("(c p) -> p c", p=bp))
                mean_b[li] = mnb

            def emit_consts(li):
                nk, dout = NK[li], DIMS[li + 1]
                sg = cpool.tile([128, nk, dout], F32, tag=f"sigma{li}")
                hw1.dma_start(sg[:], w_ls[li][:].rearrange("(k p) d -> p k d", p=128))
                nc.scalar.activation(sg[:], sg[:], AF.Exp)
                sigma[li] = sg
                mn = cpool.tile([128, nk, dout], F32, tag=f"mean{li}")
                hw1.dma_start(mn[:], w_mean[li][:].rearrange("(k p) d -> p k d", p=128))
                mean[li] = mn

            # per-sample state
            h_tiles = [dict(), dict()]
            w_tiles = dict()
            b_tiles = dict()

            def emit_bias(li, s):
                bsl = ball[li][:, s, :]
                nc.vector.tensor_mul(bsl, bsl, sigma_b[li][:])
                nc.vector.tensor_add(bsl, bsl, mean_b[li][:])
                b_tiles[(li, s)] = bsl

            def emit_wprep(li, s, dve_chunked=False, bias=True):
                nk, dout = NK[li], DIMS[li + 1]
                # sampled weights: w = z * sigma + mean (mul in place on the
                # z staging tile; the add writes the f32r weight tile -- the
                # BIR verifier requires every writer of a matmul operand's
                # memory location to produce rounded f32r)
                zt = zpool.tile([128, nk, dout], F32, tag="z")
                wt = wpools[li].tile([128, nk, dout], MMDT, tag=f"w{li}")
                hw1.dma_start(zt[:], z_w[li][s].rearrange("(k p) d -> p k d", p=128))
                ks = range(nk) if dve_chunked else [slice(None)]
                for k in ks:
                    nc.vector.tensor_mul(zt[:, k, :], zt[:, k, :], sigma[li][:, k, :])
                    nc.vector.tensor_add(wt[:, k, :], zt[:, k, :], mean[li][:, k, :])
                w_tiles[(li, s)] = wt
                if bias:
                    emit_bias(li, s)

            def get_dst(li, s):
                hp = h1p if li == 0 else h2p
                dst = hp.tile([128, NM[li], B], MMDT, tag=f"h{li}")
                h_tiles[li][s] = dst
                return dst

            def emit_matmuls(li, s, korder=False, split_last=False):
                nk, nm, mp = NK[li], NM[li], MP[li]
                wt = w_tiles.pop((li, s))
                bt = b_tiles.pop((li, s))
                src = xT_t if li == 0 else h_tiles[li - 1][s]
                if li < 2:
                    dst = get_dst(li, s)

                for m in range(nm):
                    ps = pspool.tile([mp, B], F32, tag="ps")
                    kn = (
                        [(k, n) for k in range(nk) for n in range(NB)]
                        if korder
                        else [(k, n) for n in range(NB) for k in range(nk)]
                    )
                    for k, n in kn:
                        nc.tensor.matmul(
                            ps[:, ts(n, NSLICE)],
                            wt[:, k, ts(m, mp)],
                            src[:, k, ts(n, NSLICE)],
                            start=(k == 0),
                            stop=(k == nk - 1),
                        )
                    # bank-wise eviction on boundary tiles: each PSUM bank is
                    # released right after its accumulation, so the consumer
                    # (subtile deps) starts ~3 banks earlier
                    split = split_last and m == nm - 1 and not korder
                    nslices = (
                        [ts(n, NSLICE) for n in range(NB)] if split else [slice(None)]
                    )
                    if li < 2:
                        for sl in nslices:
                            nc.scalar.activation(
                                dst[:, m, sl], ps[:, sl], AF.Tanh, bias=bt[:, m : m + 1]
                            )
                    else:
                        # output eviction on DVE (ACT is the second-busiest
                        # engine; DVE has slack)
                        osb = opool.tile([mp, B], F32, tag="osb")
                        odma = nc.scalar if s == SL - 1 else hw1
                        for sl in nslices:
                            nc.vector.tensor_scalar_add(osb[:, sl], ps[:, sl], bt[:, 0:1])
                            odma.dma_start(out_d[s][:, sl], osb[:, sl])
                        h_tiles[0].pop(s, None)
                        h_tiles[1].pop(s, None)

            # ---- PE warm-up ----
            # The PE clock is HAM-gated to 1.2 GHz until ~3.4us of sustained
            # activity. The PE would otherwise idle from its preamble (~5.5us)
            # until the first real matmul (~17us) and then run the first ~60
            # matmuls cold (427ns vs 213ns at N=512). Dummy bf16 matmuls on
            # zeroed scratch tiles warm the clock during the DMA-bound window.
            warm_w = cpool.tile([128, 128], mybir.dt.bfloat16, tag="warm_w")
            warm_x = cpool.tile([128, 256], mybir.dt.bfloat16, tag="warm_x")
            nc.gpsimd.memset(warm_w[:], 0.0)
            nc.gpsimd.memset(warm_x[:], 0.0)
            # dummies share the first real psum tile (its first real matmul
            # has start=True, which resets it) so they cost no PSUM slot
            warm_ps = pspool.tile([128, NSLICE], F32, tag="ps")
            for _ in range(18):
                nc.tensor.matmul(
                    warm_ps[:, 0:256], warm_w[:], warm_x[:], start=True, stop=True
                )

            # ---- startup: minimal critical path for layer-0 sample-0 ----
            # sync-ring order = arrival priority: ls0, z0(0), mn0, x^T
            # quarters, then sample-1 z0, then the layer-1 inputs.
            sg0 = cpool.tile([128, NK[0], DIMS[1]], F32, tag="sigma0")
            hw1.dma_start(sg0[:], w_ls[0][:].rearrange("(k p) d -> p k d", p=128))
            nc.scalar.activation(sg0[:], sg0[:], AF.Exp)
            sigma[0] = sg0
            zt0 = zpool.tile([128, NK[0], DIMS[1]], F32, tag="z")
            hw1.dma_start(zt0[:], z_w[0][0].rearrange("(k p) d -> p k d", p=128))
            mn0 = cpool.tile([128, NK[0], DIMS[1]], F32, tag="mean0")
            hw1.dma_start(mn0[:], w_mean[0][:].rearrange("(k p) d -> p k d", p=128))
            mean[0] = mn0
            # layer-0 bias items ride gpsimd SWDGE: tiny (12 KB), needed
            # early, and keeping them out of the sync ring avoids ring-full
            # back-pressure ahead of the x^T quarter transfers
            emit_bias_layer(0, dma=nc.gpsimd)
            nc.vector.tensor_mul(zt0[:], zt0[:], sg0[:])
            wt0 = wpools[0].tile([128, NK[0], DIMS[1]], MMDT, tag="w0")
            nc.vector.tensor_add(wt0[:], zt0[:], mn0[:])
            emit_bias(0, 0)

            # x^T in quarter slices (f32 staging borrows an h1 slot), cast
            # slice-wise to f32r
            xT_stage = h1p.tile([128, NK[0], B], F32, tag="h0")
            xT_t = cpool.tile([128, NK[0], B], MMDT, tag="xT")
            xT_src = xT[:].rearrange("(k p) n -> p k n", p=128)
            for n in range(NB):
                hw1.dma_start(
                    xT_stage[:, :, ts(n, NSLICE)], xT_src[:, :, ts(n, NSLICE)]
                )
                nc.vector.tensor_copy(
                    xT_t[:, :, ts(n, NSLICE)], xT_stage[:, :, ts(n, NSLICE)]
                )

            # layer-0 sample-0: batch-major single-bank tiles so matmuls
            # start as soon as the first x^T quarter is cast
            dst00 = get_dst(0, 0)
            bt00 = b_tiles.pop((0, 0))
            for n in range(NB):
                for m in range(NM[0]):
                    if n == 0 and m == 0:
                        ps = warm_ps
                    else:
                        ps = pspool.tile([MP[0], NSLICE], F32, tag="ps")
                    for k in range(NK[0]):
                        nc.tensor.matmul(
                            ps[:],
                            wt0[:, k, ts(m, MP[0])],
                            xT_t[:, k, ts(n, NSLICE)],
                            start=(k == 0),
                            stop=(k == NK[0] - 1),
                        )
                    nc.scalar.activation(
                        dst00[:, m, ts(n, NSLICE)], ps[:], AF.Tanh,
                        bias=bt00[:, m : m + 1],
                    )
            # two-sample lookahead: sample-1 layer-0 covers the window while
            # the (3 MB) layer-1 inputs stream in
            emit_wprep(0, 1)
            emit_matmuls(0, 1)
            emit_consts(1)
            emit_wprep(1, 0, dve_chunked=True, bias=False)
            emit_bias_layer(1)
            emit_bias(1, 0)
            emit_matmuls(1, 0, korder=True)
            emit_consts(2)
            emit_bias_layer(2)

            # steady state; split_last covers the only uncovered boundaries
            sched = [
                (0, 2, {}),
                (2, 0, {}),
                (1, 1, {}),
                (0, 3, {}),
                (2, 1, {}),
                (1, 2, dict(split_last=True)),
                (2, 2, {}),
                (1, 3, dict(split_last=True)),
                (2, 3, dict(split_last=True)),
            ]
            for li, s, kw in sched:
                emit_wprep(li, s)
                emit_matmuls(li, s, **kw)

    nc.compile()
    return nc


def _get_nc():
    if "nc" not in _CACHE:
        _CACHE["nc"] = _build_nc()
    return _CACHE["nc"]


def kernel(**inputs) -> np.ndarray:
    global LAST_RESULT
    nc = _get_nc()
    inp = {k: np.asarray(v, dtype=np.float32) for k, v in inputs.items()}

    xT = np.ascontiguousarray(inp["x"].T)
    in_maps = []
    for c in range(NCORES):
        sl = slice(c * SL, (c + 1) * SL)
        m = {"xT": xT}
        for li in range(3):
            m[f"w_mean_{li}"] = inp[f"w_mean_{li}"]
            m[f"w_log_std_{li}"] = inp[f"w_log_std_{li}"]
            m[f"b_mean_{li}"] = inp[f"b_mean_{li}"]
            m[f"b_log_std_{li}"] = inp[f"b_log_std_{li}"]
            m[f"z_w_{li}"] = np.ascontiguousarray(inp[f"z_w_{li}"][sl])
            m[f"z_b_{li}"] = np.ascontiguousarray(inp[f"z_b_{li}"][sl, 0, :])
        in_maps.append(m)

    res = bass_utils.run_bass_kernel_spmd(
        nc, in_maps, core_ids=list(range(NCORES)), **RUN_KWARGS
    )
    LAST_RESULT = res
    full = np.concatenate([res.results[c]["out"] for c in range(NCORES)], axis=0)
    return np.ascontiguousarray(full.transpose(0, 2, 1)).astype(np.float32)


# revision 56
# speedup vs baseline: 1.0129x; 1.0129x over previous
"""Bayesian MLP MC-sample kernel for one TRN2 chip (8 NeuronCores).

Problem: out[s, b, o] for S=32 MC samples of a 3-layer MLP
  dims 256 -> 512 -> 512 -> 64, batch B=2048,
  w_s = z_w[s] * exp(w_log_std) + w_mean   (per-sample reparameterized weights)
  h1 = tanh(x @ w0_s + b0_s); h2 = tanh(h1 @ w1_s + b1_s); out = h2 @ w2_s + b2_s

Sharding: MC-sample axis across the 8 cores (4 samples/core); x and the
mean/log_std parameters are replicated. No cross-core communication.

On-chip layout: everything stays transposed (features on partitions,
batch on the free dim) so the matmul contraction is always the partition
dim and no transposes are needed on device:
  h^T[dout, B] = W^T x^T   via  matmul(psum, lhsT=w[k_chunk, dout_chunk],
                                       rhs=hprev^T[k_chunk, batch_slice])
The host passes x^T in and transposes the (S, 64, B) result back to
(S, B, 64) when gathering -- pure layout work, no FLOPs.

Per-core engine usage:
  PE:  4 samples x 112 matmuls (N=512, f32r -> 1 cycle/row)
  ACT: exp(log_std) once; per-sample tanh(psum + bias) eviction
  DVE: per-sample w = z * sigma + mean (two passes, in place)
  DMA: z shards + replicated params + x^T in, out^T back
"""

import numpy as np

import concourse.bass as bass
import concourse.mybir as mybir
import concourse.tile as tile
from concourse import bacc
from concourse import bass_utils

F32 = mybir.dt.float32
F32R = mybir.dt.float32r
# matmul compute dtype: f32r (TF32-like, 1 cycle/row at N>=256) gives
# 2.4e-4 end-to-end rel err vs 3.9e-3 for bf16 at only ~4% more runtime
MMDT = F32R
AF = mybir.ActivationFunctionType
ts = bass.ts

S = 32
B = 2048
DIMS = [256, 512, 512, 64]
NCORES = 8
SL = S // NCORES  # samples per core
NSLICE = 512      # moving-dim slice (max for fp32 matmul, = 1 PSUM bank)
NB = B // NSLICE

# knobs test.py may override before the first kernel() call
RUN_KWARGS: dict = {}
LAST_RESULT = None

_CACHE: dict = {}


def _build_nc():
    nc = bacc.Bacc("TRN2", target_bir_lowering=False)

    xT = nc.dram_tensor("xT", [DIMS[0], B], F32, kind="ExternalInput")
    w_mean, w_ls, b_mean, b_ls, z_w, z_b = [], [], [], [], [], []
    for li in range(3):
        din, dout = DIMS[li], DIMS[li + 1]
        w_mean.append(nc.dram_tensor(f"w_mean_{li}", [din, dout], F32, kind="ExternalInput"))
        w_ls.append(nc.dram_tensor(f"w_log_std_{li}", [din, dout], F32, kind="ExternalInput"))
        b_mean.append(nc.dram_tensor(f"b_mean_{li}", [dout], F32, kind="ExternalInput"))
        b_ls.append(nc.dram_tensor(f"b_log_std_{li}", [dout], F32, kind="ExternalInput"))
        z_w.append(nc.dram_tensor(f"z_w_{li}", [SL, din, dout], F32, kind="ExternalInput"))
        z_b.append(nc.dram_tensor(f"z_b_{li}", [SL, dout], F32, kind="ExternalInput"))
    out_d = nc.dram_tensor("out", [SL, DIMS[3], B], F32, kind="ExternalOutput")

    NK = [d // 128 for d in DIMS[:3]]      # k-chunks per layer: 2, 4, 4
    MP = [min(128, d) for d in DIMS[1:]]   # psum partitions:  128, 128, 64
    NM = [d // 128 if d >= 128 else 1 for d in DIMS[1:]]  # m-chunks: 4, 4, 1
    BP = [min(128, d) for d in DIMS[1:]]   # bias partitions
    BC = [max(1, d // 128) for d in DIMS[1:]]  # bias cols

    with tile.TileContext(nc) as tc:
        with (
            tc.tile_pool(name="const", bufs=1) as cpool,
            tc.tile_pool(name="z", bufs=2) as zpool,
            tc.tile_pool(name="w0", bufs=2) as w0p,
            tc.tile_pool(name="w1", bufs=2) as w1p,
            tc.tile_pool(name="w2", bufs=2) as w2p,
            tc.tile_pool(name="h1", bufs=2) as h1p,
            tc.tile_pool(name="h2", bufs=1) as h2p,
            tc.tile_pool(name="osb", bufs=1) as opool,
            tc.tile_pool(name="ps", bufs=2, space="PSUM") as pspool,
        ):
            wpools = [w0p, w1p, w2p]
            # All transfers ride the sync-engine HWDGE ring: a large
            # dma_start spreads over all 16 SDMA engines (~350 GB/s), and
            # the ring serves transfers in issue order, which doubles as
            # the prefetch priority. (Measured: gpsimd SWDGE ~20 GB/s and
            # scalar-ring issues stall the ACT FIFO -- both hurt; small
            # transfers cost ~1.5-2 us of ring dead time each, hence the
            # per-layer batching of bias vectors.)
            hw1 = nc.sync
            sw = nc.sync

            sigma = [None] * 3
            mean = [None] * 3
            sigma_b = [None] * 3
            mean_b = [None] * 3

            ball = [None] * 3

            def emit_bias_layer(li, dma=None):
                # all SL samples' bias noise in ONE transfer -- small DMAs
                # cost ~1.5 us of ring dead time each
                dma = dma or sw
                bp, bc = BP[li], BC[li]
                bz = cpool.tile([bp, SL, bc], F32, tag=f"ball{li}")
                dma.dma_start(bz[:], z_b[li][:].rearrange("s (c p) -> p s c", p=bp))
                ball[li] = bz
                sgb = cpool.tile([bp, bc], F32, tag=f"sigma_b{li}")
                dma.dma_start(sgb[:], b_ls[li][:].rearrange("(c p) -> p c", p=bp))
                nc.scalar.activation(sgb[:], sgb[:], AF.Exp)
                sigma_b[li] = sgb
                mnb = cpool.tile([bp, bc], F32, tag=f"mean_b{li}")
                dma.dma_start(mnb[:], b_mean[li][:].rearrange("(c p) -> p c", p=bp))
                mean_b[li] = mnb

            def emit_consts(li):
                nk, dout = NK[li], DIMS[li + 1]
                sg = cpool.tile([128, nk, dout], F32, tag=f"sigma{li}")
                hw1.dma_start(sg[:], w_ls[li][:].rearrange("(k p) d -> p k d", p=128))
                nc.scalar.activation(sg[:], sg[:], AF.Exp)
                sigma[li] = sg
                mn = cpool.tile([128, nk, dout], F32, tag=f"mean{li}")
                hw1.dma_start(mn[:], w_mean[li][:].rearrange("(k p) d -> p k d", p=128))
                mean[li] = mn

            # per-sample state
            h_tiles = [dict(), dict()]
            w_tiles = dict()
            b_tiles = dict()

            def emit_bias(li, s):
                bsl = ball[li][:, s, :]
                nc.vector.tensor_mul(bsl, bsl, sigma_b[li][:])
                nc.vector.tensor_add(bsl, bsl, mean_b[li][:])
                b_tiles[(li, s)] = bsl

            def emit_wprep(li, s, dve_chunked=False, bias=True):
                nk, dout = NK[li], DIMS[li + 1]
                # sampled weights: w = z * sigma + mean (mul in place on the
                # z staging tile; the add writes the f32r weight tile -- the
                # BIR verifier requires every writer of a matmul operand's
                # memory location to produce rounded f32r)
                zt = zpool.tile([128, nk, dout], F32, tag="z")
                wt = wpools[li].tile([128, nk, dout], MMDT, tag=f"w{li}")
                hw1.dma_start(zt[:], z_w[li][s].rearrange("(k p) d -> p k d", p=128))
                ks = range(nk) if dve_chunked else [slice(None)]
                for k in ks:
                    nc.vector.tensor_mul(zt[:, k, :], zt[:, k, :], sigma[li][:, k, :])
                    nc.vector.tensor_add(wt[:, k, :], zt[:, k, :], mean[li][:, k, :])
                w_tiles[(li, s)] = wt
                if bias:
                    emit_bias(li, s)

            def get_dst(li, s):
                hp = h1p if li == 0 else h2p
                dst = hp.tile([128, NM[li], B], MMDT, tag=f"h{li}")
                h_tiles[li][s] = dst
                return dst

            def emit_matmuls(li, s, korder=False, split_last=False):
                nk, nm, mp = NK[li], NM[li], MP[li]
                wt = w_tiles.pop((li, s))
                bt = b_tiles.pop((li, s))
                src = xT_t if li == 0 else h_tiles[li - 1][s]
                if li < 2:
                    dst = get_dst(li, s)

                for m in range(nm):
                    ps = pspool.tile([mp, B], F32, tag="ps")
                    kn = (
                        [(k, n) for k in range(nk) for n in range(NB)]
                        if korder
                        else [(k, n) for n in range(NB) for k in range(nk)]
                    )
                    for k, n in kn:
                        nc.tensor.matmul(
                            ps[:, ts(n, NSLICE)],
                            wt[:, k, ts(m, mp)],
                            src[:, k, ts(n, NSLICE)],
                            start=(k == 0),
                            stop=(k == nk - 1),
                        )
                    # bank-wise eviction on boundary tiles: each PSUM bank is
                    # released right after its accumulation, so the consumer
                    # (subtile deps) starts ~3 banks earlier
                    split = split_last and m == nm - 1 and not korder
                    nslices = (
                        [ts(n, NSLICE) for n in range(NB)] if split else [slice(None)]
                    )
                    if li < 2:
                        for sl in nslices:
                            nc.scalar.activation(
                                dst[:, m, sl], ps[:, sl], AF.Tanh, bias=bt[:, m : m + 1]
                            )
                    else:
                        # output eviction on DVE (ACT is the second-busiest
                        # engine; DVE has slack)
                        osb = opool.tile([mp, B], F32, tag="osb")
                        odma = nc.scalar if s == SL - 1 else hw1
                        for sl in nslices:
                            nc.vector.tensor_scalar_add(osb[:, sl], ps[:, sl], bt[:, 0:1])
                            odma.dma_start(out_d[s][:, sl], osb[:, sl])
                        h_tiles[0].pop(s, None)
                        h_tiles[1].pop(s, None)

            # ---- PE warm-up ----
            # The PE clock is HAM-gated to 1.2 GHz until ~3.4us of sustained
            # activity. The PE would otherwise idle from its preamble (~5.5us)
            # until the first real matmul (~17us) and then run the first ~60
            # matmuls cold (427ns vs 213ns at N=512). Dummy bf16 matmuls on
            # zeroed scratch tiles warm the clock during the DMA-bound window.
            warm_w = cpool.tile([128, 128], mybir.dt.bfloat16, tag="warm_w")
            warm_x = cpool.tile([128, NSLICE], mybir.dt.bfloat16, tag="warm_x")
            nc.gpsimd.memset(warm_w[:], 0.0)
            nc.gpsimd.memset(warm_x[:], 0.0)
            # dummies share the first real psum tile (its first real matmul
            # has start=True, which resets it) so they cost no PSUM slot
            warm_ps = pspool.tile([128, NSLICE], F32, tag="ps")
            for _ in range(20):
                nc.tensor.matmul(warm_ps[:], warm_w[:], warm_x[:], start=True, stop=True)

            # ---- startup: minimal critical path for layer-0 sample-0 ----
            # sync-ring order = arrival priority: ls0, z0(0), mn0, x^T
            # quarters, then sample-1 z0, then the layer-1 inputs.
            sg0 = cpool.tile([128, NK[0], DIMS[1]], F32, tag="sigma0")
            hw1.dma_start(sg0[:], w_ls[0][:].rearrange("(k p) d -> p k d", p=128))
            nc.scalar.activation(sg0[:], sg0[:], AF.Exp)
            sigma[0] = sg0
            zt0 = zpool.tile([128, NK[0], DIMS[1]], F32, tag="z")
            hw1.dma_start(zt0[:], z_w[0][0].rearrange("(k p) d -> p k d", p=128))
            mn0 = cpool.tile([128, NK[0], DIMS[1]], F32, tag="mean0")
            hw1.dma_start(mn0[:], w_mean[0][:].rearrange("(k p) d -> p k d", p=128))
            mean[0] = mn0
            # layer-0 bias items ride gpsimd SWDGE: tiny (12 KB), needed
            # early, and keeping them out of the sync ring avoids ring-full
            # back-pressure ahead of the x^T quarter transfers
            emit_bias_layer(0, dma=nc.gpsimd)
            nc.vector.tensor_mul(zt0[:], zt0[:], sg0[:])
            wt0 = wpools[0].tile([128, NK[0], DIMS[1]], MMDT, tag="w0")
            nc.vector.tensor_add(wt0[:], zt0[:], mn0[:])
            emit_bias(0, 0)

            # x^T in quarter slices (f32 staging borrows an h1 slot), cast
            # slice-wise to f32r
            xT_stage = h1p.tile([128, NK[0], B], F32, tag="h0")
            xT_t = cpool.tile([128, NK[0], B], MMDT, tag="xT")
            xT_src = xT[:].rearrange("(k p) n -> p k n", p=128)
            for n in range(NB):
                hw1.dma_start(
                    xT_stage[:, :, ts(n, NSLICE)], xT_src[:, :, ts(n, NSLICE)]
                )
                nc.vector.tensor_copy(
                    xT_t[:, :, ts(n, NSLICE)], xT_stage[:, :, ts(n, NSLICE)]
                )

            # layer-0 sample-0: batch-major single-bank tiles so matmuls
            # start as soon as the first x^T quarter is cast
            dst00 = get_dst(0, 0)
            bt00 = b_tiles.pop((0, 0))
            for n in range(NB):
                for m in range(NM[0]):
                    if n == 0 and m == 0:
                        ps = warm_ps
                    else:
                        ps = pspool.tile([MP[0], NSLICE], F32, tag="ps")
                    for k in range(NK[0]):
                        nc.tensor.matmul(
                            ps[:],
                            wt0[:, k, ts(m, MP[0])],
                            xT_t[:, k, ts(n, NSLICE)],
                            start=(k == 0),
                            stop=(k == NK[0] - 1),
                        )
                    nc.scalar.activation(
                        dst00[:, m, ts(n, NSLICE)], ps[:], AF.Tanh,
                        bias=bt00[:, m : m + 1],
                    )
            # two-sample lookahead: sample-1 layer-0 covers the window while
            # the (3 MB) layer-1 inputs stream in
            emit_wprep(0, 1)
            emit_matmuls(0, 1)
            emit_consts(1)
            emit_wprep(1, 0, dve_chunked=True, bias=False)
            emit_bias_layer(1)
            emit_bias(1, 0)
            emit_matmuls(1, 0, korder=True)
            emit_consts(2)
            emit_bias_layer(2)

            # steady state; split_last covers the only uncovered boundaries
            sched = [
                (0, 2, {}),
                (2, 0, {}),
                (1, 1, {}),
                (0, 3, {}),
                (2, 1, {}),
                (1, 2, dict(split_last=True)),
                (2, 2, {}),
                (1, 3, dict(split_last=True)),
                (2, 3, dict(split_last=True)),
            ]
            for li, s, kw in sched:
                emit_wprep(li, s)
                emit_matmuls(li, s, **kw)

    nc.compile()
    return nc


def _get_nc():
    if "nc" not in _CACHE:
        _CACHE["nc"] = _build_nc()
    return _CACHE["nc"]


def kernel(**inputs) -> np.ndarray:
    global LAST_RESULT
    nc = _get_nc()
    inp = {k: np.asarray(v, dtype=np.float32) for k, v in inputs.items()}

    xT = np.ascontiguousarray(inp["x"].T)
    in_maps = []
    for c in range(NCORES):
        sl = slice(c * SL, (c + 1) * SL)
        m = {"xT": xT}
        for li in range(3):
            m[f"w_mean_{li}"] = inp[f"w_mean_{li}"]
            m[f"w_log_std_{li}"] = inp[f"w_log_std_{li}"]
            m[f"b_mean_{li}"] = inp[f"b_mean_{li}"]
            m[f"b_log_std_{li}"] = inp[f"b_log_std_{li}"]
            m[f"z_w_{li}"] = np.ascontiguousarray(inp[f"z_w_{li}"][sl])
            m[f"z_b_{li}"] = np.ascontiguousarray(inp[f"z_b_{li}"][sl, 0, :])
        in_maps.append(m)

    res = bass_utils.run_bass_kernel_spmd(
        nc, in_maps, core_ids=list(range(NCORES)), **RUN_KWARGS
    )
    LAST_RESULT = res
    full = np.concatenate([res.results[c]["out"] for c in range(NCORES)], axis=0)
    return np.ascontiguousarray(full.transpose(0, 2, 1)).astype(np.float32)
